# revision 23
# baseline (speedup 1.0000x reference)
"""Trainium2 Bass kernel for the SE-gated Non-local block (rank-1 attention).

Math (per batch item b, x viewed as [C, N] with N = H*W):
    S[c]    = sum_n x[c, n]                      (spatial sum)
    hid     = relu((se_w1 / N) @ S + se_b1)      (SE bottleneck; 1/N folds the mean)
    gate    = sigmoid(se_w2 @ hid + se_b2)       [C]
    w3e     = gate * [g_w | theta_w | phi_w]     [C, 3]   (gate folded into projections)
    proj    = w3e.T @ x                          [3, N]   (rows: g, theta, phi;
                                                 the 1x1-conv biases are zero)
    s_raw   = sum_n proj[0] * proj[2]
    out     = x + (A * s_raw) (outer) theta + Bc (outer) ones   where
              inv = bn_gamma / sqrt(bn_var + eps)
              A   = W_w * inv / N                (1/N folds the f/N normalizer)
              Bc  = (W_b - bn_mean) * inv + bn_beta

Memory-roofline design: the only mandatory HBM traffic is read-x + write-out
(37.7 MB/core at 358 GB/s ~ 105 us).  To keep the DMA rings saturated the
whole time, x is held in SBUF as *bf16* (4.7 MB/item instead of 9.4), so both
batch items of a core fit on-chip and all 8 chunk loads stream back-to-back
while item 0's gate/projection chain overlaps item 1's loads, and item 0's
stores overlap item 1's chain.  The f32->bf16 cast happens inside the load
DMA (SWDGE) and the bf16->f32 cast inside the store DMA, so no engine pass
touches the bulk data except the single in-place correction add.

Precision: out = bf16(x) + correction, quantized to bf16 before the store
cast.  That costs ~2e-3 output rel err (bf16 mantissa), well inside the
2e-2 gate; the correction term itself (rms ~5e-6 vs |x| ~ 1) runs in bf16
end-to-end.

Engine budget (per item, vs the ~26 us load/store window it must hide in):
  DVE    ~18 us: item-0 spatial sums (1x reduce), proj psum combines,
                 w3e, dot, 2 affines (4x), 4 in-place adds (2x)
  ACT    ~12 us: SE relu/sigmoid, 9 ubt copies, 2 affines
  GpSimd       : item-1 spatial sums (streams behind the loads) + SWDGE
                 descriptor generation for the cast loads/stores
  PE     ~10 us: SE matmuls, proj (2 psum banks in parallel), theta
                 broadcast (selector matmul), cross-partition ones-matmul

Queue layout: x loads + out stores on the gpsimd SWDGE ring (they need the
dtype cast), weight loads + the g/phi row bounce on the sync HWDGE ring.
Sharding: pure data parallel, 2 of the 16 batch items per core, params
replicated, no collectives.
"""

import numpy as np

B, C, H, W = 16, 512, 96, 48
N = H * W            # 4608
P = 128
KC = C // P          # 4 channel chunks
NB = 512             # free-dim block = one fp32 PSUM bank
NJ = N // NB         # 9
NCORES = 8
BPC = B // NCORES    # 2 batch items per core
SE_C = C // 16       # 32
BN_EPS = 1e-5

_CACHE = {}
LAST_RESULTS = None


def _build_bass():
    import concourse.mybir as mybir
    from concourse.bacc import Bacc
    from concourse.tile import TileContext

    f32 = mybir.dt.float32
    bf16 = mybir.dt.bfloat16
    AF = mybir.ActivationFunctionType
    AX = mybir.AxisListType
    ALU = mybir.AluOpType

    nc = Bacc()
    xs = nc.dram_tensor("xs", [BPC, C, N], f32, kind="ExternalInput")
    w1 = nc.dram_tensor("w1", [P, KC, SE_C], f32, kind="ExternalInput")
    w2 = nc.dram_tensor("w2", [SE_C, C], f32, kind="ExternalInput")
    b1 = nc.dram_tensor("b1", [SE_C, 1], f32, kind="ExternalInput")
    b2 = nc.dram_tensor("b2", [P, KC], f32, kind="ExternalInput")
    w3 = nc.dram_tensor("w3", [P, KC, 3], bf16, kind="ExternalInput")
    at = nc.dram_tensor("at", [P, KC], f32, kind="ExternalInput")   # A chunks
    bc = nc.dram_tensor("bc", [P, KC], f32, kind="ExternalInput")   # Bc chunks
    sel = nc.dram_tensor("sel", [3, P], bf16, kind="ExternalInput")  # theta row
    out_d = nc.dram_tensor("out", [BPC, C, N], f32, kind="ExternalOutput")
    tp_scr = nc.dram_tensor("tp_scr", [BPC, 3, N], bf16)

    MR = N // P  # 36: elements per partition in the reshaped g/phi rows

    with TileContext(nc) as tc:
        with (
            tc.tile_pool(name="wpool", bufs=1) as wpool,
            tc.tile_pool(name="xpool", bufs=2 * KC) as xpool,
            tc.tile_pool(name="ppool", bufs=2) as ppool,
            tc.tile_pool(name="spool", bufs=2) as spool,
            tc.tile_pool(name="tpool", bufs=2) as tpool,
            tc.tile_pool(name="ps_se", bufs=2, space="PSUM") as ps_se,
            tc.tile_pool(name="ps_pp", bufs=3, space="PSUM") as ps_pp,
            tc.tile_pool(name="ps_ub", bufs=2, space="PSUM") as ps_ub,
        ):
            w1t = wpool.tile([P, KC, SE_C], f32, tag="w1t")
            w2t = wpool.tile([SE_C, C], f32, tag="w2t")
            b1t = wpool.tile([SE_C, 1], f32, tag="b1t")
            b2t = wpool.tile([P, KC], f32, tag="b2t")
            w3t = wpool.tile([P, KC, 3], bf16, tag="w3t")
            att = wpool.tile([P, KC], f32, tag="att")
            bct = wpool.tile([P, KC], f32, tag="bct")
            selt = wpool.tile([3, P], bf16, tag="selt")
            on128 = wpool.tile([P, P], f32, tag="on128")  # all-ones (part. sum)
            warm = wpool.tile([P, 1], f32, tag="warm")

            # weights on the sync ring so the gpsimd ring is free for x loads
            nc.vector.memset(on128[:], 1.0)
            # pull the sigmoid ACT table load out of the critical chain
            nc.scalar.activation(out=warm[:], in_=on128[:, 0:1],
                                 func=AF.Sigmoid)
            for t, d in ((w1t, w1), (w2t, w2), (b1t, b1), (b2t, b2),
                         (w3t, w3), (att, at), (bct, bc), (selt, sel)):
                nc.sync.dma_start(out=t[:], in_=d[:])

            # ---- all 8 x-chunk loads enqueued upfront (SWDGE f32->bf16) ----
            xbs = [[None] * KC for _ in range(BPC)]
            for b in range(BPC):
                for k in range(KC):
                    xb = xpool.tile([P, N], bf16, tag="xb")
                    nc.gpsimd.dma_start(out=xb[:], in_=xs[b, k * P:(k + 1) * P, :])
                    xbs[b][k] = xb

            # ---- spatial sums, streaming behind the loads.  Item 0 on DVE
            #      (idle during the load phase), item 1 on ACT (in-place
            #      identity, sum on the accumulate port) -- splitting across
            #      engines keeps either from serializing the other item's
            #      chain; GpSimd/walrus support neither free-dim reduce nor
            #      the accum rider ----
            xps = []
            for b in range(BPC):
                xp = spool.tile([P, KC], f32, tag="xp")
                for k in range(KC):
                    if b == 0:
                        nc.vector.reduce_sum(out=xp[:, k:k + 1],
                                             in_=xbs[b][k][:], axis=AX.X)
                    else:
                        nc.scalar.activation(out=xbs[b][k][:],
                                             in_=xbs[b][k][:],
                                             func=AF.Identity,
                                             accum_out=xp[:, k:k + 1])
                xps.append(xp)

            for b in range(BPC):
                xp = xps[b]
                # ---- SE gate ----
                php = ps_se.tile([SE_C, 1], f32, tag="ps_se")
                for k in range(KC):
                    nc.tensor.matmul(php[:], w1t[:, k, :], xp[:, k:k + 1],
                                     start=(k == 0), stop=(k == KC - 1))
                hid = spool.tile([SE_C, 1], f32, tag="hid")
                nc.scalar.activation(out=hid[:], in_=php[:], func=AF.Relu,
                                     bias=b1t[:], scale=1.0)

                gate = spool.tile([P, KC], f32, tag="gate")
                for k in range(KC):
                    gp = ps_se.tile([P, 1], f32, tag="ps_se")
                    nc.tensor.matmul(gp[:], w2t[:, k * P:(k + 1) * P], hid[:],
                                     start=True, stop=True)
                    nc.scalar.activation(out=gate[:, k:k + 1], in_=gp[:],
                                         func=AF.Sigmoid, bias=b2t[:, k:k + 1],
                                         scale=1.0)

                # ---- gated projections: proj = w3e.T @ x (bf16 PE) ----
                w3e = spool.tile([P, KC, 3], bf16, tag="w3e")
                for k in range(KC):
                    nc.vector.tensor_scalar_mul(out=w3e[:, k, :],
                                                in0=w3t[:, k, :],
                                                scalar1=gate[:, k:k + 1])

                proj = ppool.tile([3, N], bf16, tag="proj")
                ubt = tpool.tile([P, N], bf16, tag="ubt")
                for j in range(NJ):
                    sl = slice(j * NB, (j + 1) * NB)
                    pp = ps_pp.tile([3, NB], f32, tag="pp")
                    for k in range(KC):
                        nc.tensor.matmul(pp[:], w3e[:, k, :], xbs[b][k][:, sl],
                                         start=(k == 0), stop=(k == KC - 1))
                    nc.vector.tensor_copy(out=proj[:, sl], in_=pp[:])
                    # theta -> all 128 partitions, on-chip: selector matmul
                    ub_ps = ps_ub.tile([P, NB], f32, tag="ub_ps")
                    nc.tensor.matmul(ub_ps[:], selt[:], proj[:, sl],
                                     start=True, stop=True)
                    nc.scalar.activation(out=ubt[:, sl], in_=ub_ps[:],
                                         func=AF.Identity, scale=1.0)

                # ---- g,phi -> [128, 36] (DRAM bounce) so the dot product
                #      uses every DVE lane ----
                g_rs = spool.tile([P, MR], bf16, tag="g_rs")
                p_rs = spool.tile([P, MR], bf16, tag="p_rs")
                nc.sync.dma_start(out=tp_scr[b], in_=proj[:])
                nc.sync.dma_start(
                    out=g_rs[:],
                    in_=tp_scr[b, 0, :].rearrange("(p m) -> p m", p=P))
                nc.sync.dma_start(
                    out=p_rs[:],
                    in_=tp_scr[b, 2, :].rearrange("(p m) -> p m", p=P))

                # ---- s_raw = <g, phi>; ast = A * s_raw ----
                prod = spool.tile([P, MR], f32, tag="prod")
                nc.vector.tensor_mul(out=prod[:], in0=g_rs[:], in1=p_rs[:])
                r1 = spool.tile([P, 1], f32, tag="r1")
                nc.vector.reduce_sum(out=r1[:], in_=prod[:], axis=AX.X)
                sb = ps_se.tile([P, 1], f32, tag="ps_se")
                nc.tensor.matmul(sb[:], on128[:], r1[:], start=True, stop=True)
                ast = spool.tile([P, KC], f32, tag="ast")
                nc.vector.tensor_scalar_mul(out=ast[:], in0=att[:],
                                            scalar1=sb[:])

                # ---- out = bf16(x) + (A*s)*theta + Bc, in place; store with
                #      bf16->f32 cast in the DMA.  Affine at 4x + add at 2x,
                #      both DVE (ACT is busy with item 1's sums here) ----
                for k in range(KC):
                    t1 = tpool.tile([P, N], bf16, tag="t1")
                    nc.vector.tensor_scalar(out=t1[:], in0=ubt[:],
                                            scalar1=ast[:, k:k + 1],
                                            scalar2=bct[:, k:k + 1],
                                            op0=ALU.mult, op1=ALU.add)
                    nc.vector.tensor_add(out=xbs[b][k][:],
                                         in0=xbs[b][k][:], in1=t1[:])
                    nc.gpsimd.dma_start(out=out_d[b, k * P:(k + 1) * P, :],
                                        in_=xbs[b][k][:])

    nc.finalize()  # runs Bacc compile passes (wait splitting, reg alloc, ...)
    return nc


def kernel(**inputs):
    global LAST_RESULTS
    from concourse.bass_utils import run_bass_kernel_spmd

    a = {k: np.asarray(v, dtype=np.float32) for k, v in inputs.items()}
    x = np.ascontiguousarray(a["x"]).reshape(B, C, N)

    inv = a["bn_gamma"] / np.sqrt(a["bn_var"] + BN_EPS)
    A = (a["W_w"] * inv / N).astype(np.float32)
    Bc = ((a["W_b"] - a["bn_mean"]) * inv + a["bn_beta"]).astype(np.float32)

    # the kernel folds the (always-zero) g/theta/phi conv biases away
    assert abs(float(a["g_b"])) < 1e-30 and abs(float(a["theta_b"])) < 1e-30 \
        and abs(float(a["phi_b"])) < 1e-30, "nonzero projection bias"

    w1h = np.ascontiguousarray(
        (a["se_w1"] / N).T.reshape(KC, P, SE_C).transpose(1, 0, 2)).astype(np.float32)
    w2h = np.ascontiguousarray(a["se_w2"].T).astype(np.float32)
    b1h = np.ascontiguousarray(a["se_b1"].reshape(SE_C, 1))
    b2h = np.ascontiguousarray(a["se_b2"].reshape(KC, P).T)
    import ml_dtypes
    w3h = np.ascontiguousarray(
        np.stack([a["g_w"], a["theta_w"], a["phi_w"]], axis=1)
        .reshape(KC, P, 3).transpose(1, 0, 2)).astype(ml_dtypes.bfloat16)
    ath = np.ascontiguousarray(A.reshape(KC, P).T)
    bch = np.ascontiguousarray(Bc.reshape(KC, P).T)
    selh = np.zeros((3, P), dtype=ml_dtypes.bfloat16)
    selh[1, :] = 1.0

    if "nc" not in _CACHE:
        _CACHE["nc"] = _build_bass()
    nc = _CACHE["nc"]

    in_maps = []
    for c in range(NCORES):
        in_maps.append({
            "xs": np.ascontiguousarray(x[c * BPC:(c + 1) * BPC]),
            "w1": w1h, "w2": w2h, "b1": b1h, "b2": b2h,
            "w3": w3h, "at": ath, "bc": bch, "sel": selh,
        })

    res = run_bass_kernel_spmd(nc, in_maps, core_ids=list(range(NCORES)))
    LAST_RESULTS = res

    out = np.concatenate([res.results[c]["out"] for c in range(NCORES)], axis=0)
    return np.ascontiguousarray(out.reshape(B, C, H, W))


# revision 24
# speedup vs baseline: 1.0287x; 1.0287x over previous
"""Trainium2 Bass kernel for the SE-gated Non-local block (rank-1 attention).

Math (per batch item b, x viewed as [C, N] with N = H*W):
    S[c]    = sum_n x[c, n]                      (spatial sum)
    hid     = relu((se_w1 / N) @ S + se_b1)      (SE bottleneck; 1/N folds the mean)
    gate    = sigmoid(se_w2 @ hid + se_b2)       [C]
    w3e     = gate * [g_w | theta_w | phi_w]     [C, 3]   (gate folded into projections)
    proj    = w3e.T @ x                          [3, N]   (rows: g, theta, phi;
                                                 the 1x1-conv biases are zero)
    s_raw   = sum_n proj[0] * proj[2]
    out     = x + (A * s_raw) (outer) theta + Bc (outer) ones   where
              inv = bn_gamma / sqrt(bn_var + eps)
              A   = W_w * inv / N                (1/N folds the f/N normalizer)
              Bc  = (W_b - bn_mean) * inv + bn_beta

Memory-roofline design: the only mandatory HBM traffic is read-x + write-out
(37.7 MB/core at 358 GB/s ~ 105 us).  To keep the DMA rings saturated the
whole time, x is held in SBUF as *bf16* (4.7 MB/item instead of 9.4), so both
batch items of a core fit on-chip and all 8 chunk loads stream back-to-back
while item 0's gate/projection chain overlaps item 1's loads, and item 0's
stores overlap item 1's chain.  The f32->bf16 cast happens inside the load
DMA (SWDGE) and the bf16->f32 cast inside the store DMA, so no engine pass
touches the bulk data except the single in-place correction add.

Precision: out = bf16(x) + correction, quantized to bf16 before the store
cast.  That costs ~2e-3 output rel err (bf16 mantissa), well inside the
2e-2 gate; the correction term itself (rms ~5e-6 vs |x| ~ 1) runs in bf16
end-to-end.

Engine budget (per item, vs the ~26 us load/store window it must hide in):
  DVE    ~18 us: item-0 spatial sums (1x reduce), proj psum combines,
                 w3e, dot, 2 affines (4x), 4 in-place adds (2x)
  ACT    ~12 us: SE relu/sigmoid, 9 ubt copies, 2 affines
  GpSimd       : item-1 spatial sums (streams behind the loads) + SWDGE
                 descriptor generation for the cast loads/stores
  PE     ~10 us: SE matmuls, proj (2 psum banks in parallel), theta
                 broadcast (selector matmul), cross-partition ones-matmul

Queue layout: x loads + out stores on the gpsimd SWDGE ring (they need the
dtype cast), weight loads + the g/phi row bounce on the sync HWDGE ring.
Sharding: pure data parallel, 2 of the 16 batch items per core, params
replicated, no collectives.
"""

import numpy as np

B, C, H, W = 16, 512, 96, 48
N = H * W            # 4608
P = 128
KC = C // P          # 4 channel chunks
NB = 512             # free-dim block = one fp32 PSUM bank
NJ = N // NB         # 9
NCORES = 8
BPC = B // NCORES    # 2 batch items per core
SE_C = C // 16       # 32
BN_EPS = 1e-5

_CACHE = {}
LAST_RESULTS = None


def _build_bass():
    import concourse.mybir as mybir
    from concourse.bacc import Bacc
    from concourse.tile import TileContext

    f32 = mybir.dt.float32
    bf16 = mybir.dt.bfloat16
    AF = mybir.ActivationFunctionType
    AX = mybir.AxisListType
    ALU = mybir.AluOpType

    nc = Bacc()
    xs = nc.dram_tensor("xs", [BPC, C, N], f32, kind="ExternalInput")
    w1 = nc.dram_tensor("w1", [P, KC, SE_C], f32, kind="ExternalInput")
    w2 = nc.dram_tensor("w2", [SE_C, C], f32, kind="ExternalInput")
    b1 = nc.dram_tensor("b1", [SE_C, 1], f32, kind="ExternalInput")
    b2 = nc.dram_tensor("b2", [P, KC], f32, kind="ExternalInput")
    w3 = nc.dram_tensor("w3", [P, KC, 3], bf16, kind="ExternalInput")
    at = nc.dram_tensor("at", [P, KC], f32, kind="ExternalInput")   # A chunks
    bc = nc.dram_tensor("bc", [P, KC], f32, kind="ExternalInput")   # Bc chunks
    sel = nc.dram_tensor("sel", [3, P], bf16, kind="ExternalInput")  # theta row
    out_d = nc.dram_tensor("out", [BPC, C, N], f32, kind="ExternalOutput")
    tp_scr = nc.dram_tensor("tp_scr", [BPC, 3, N], bf16)

    MR = N // P  # 36: elements per partition in the reshaped g/phi rows

    with TileContext(nc) as tc:
        with (
            tc.tile_pool(name="wpool", bufs=1) as wpool,
            tc.tile_pool(name="xpool", bufs=2 * KC) as xpool,
            tc.tile_pool(name="ppool", bufs=2) as ppool,
            tc.tile_pool(name="spool", bufs=2) as spool,
            tc.tile_pool(name="tpool", bufs=2) as tpool,
            tc.tile_pool(name="ps_se", bufs=2, space="PSUM") as ps_se,
            tc.tile_pool(name="ps_pp", bufs=3, space="PSUM") as ps_pp,
            tc.tile_pool(name="ps_ub", bufs=2, space="PSUM") as ps_ub,
        ):
            w1t = wpool.tile([P, KC, SE_C], f32, tag="w1t")
            w2t = wpool.tile([SE_C, C], f32, tag="w2t")
            b1t = wpool.tile([SE_C, 1], f32, tag="b1t")
            b2t = wpool.tile([P, KC], f32, tag="b2t")
            w3t = wpool.tile([P, KC, 3], bf16, tag="w3t")
            att = wpool.tile([P, KC], f32, tag="att")
            bct = wpool.tile([P, KC], f32, tag="bct")
            selt = wpool.tile([3, P], bf16, tag="selt")
            on128 = wpool.tile([P, P], f32, tag="on128")  # all-ones (part. sum)
            warm = wpool.tile([P, 1], f32, tag="warm")

            # weights on the sync ring so the gpsimd ring is free for x loads
            nc.vector.memset(on128[:], 1.0)
            # pull the sigmoid ACT table load out of the critical chain
            nc.scalar.activation(out=warm[:], in_=on128[:, 0:1],
                                 func=AF.Sigmoid)
            for t, d in ((w1t, w1), (w2t, w2), (b1t, b1), (b2t, b2),
                         (w3t, w3), (att, at), (bct, bc), (selt, sel)):
                nc.sync.dma_start(out=t[:], in_=d[:])

            # ---- all 8 x-chunk loads enqueued upfront (SWDGE f32->bf16) ----
            xbs = [[None] * KC for _ in range(BPC)]
            for b in range(BPC):
                for k in range(KC):
                    xb = xpool.tile([P, N], bf16, tag="xb")
                    nc.gpsimd.dma_start(out=xb[:], in_=xs[b, k * P:(k + 1) * P, :])
                    xbs[b][k] = xb

            for b in range(BPC):
                # ---- spatial sums, streaming behind the loads.  Item 0 on
                #      DVE (idle then), item 1 on ACT (in-place identity,
                #      sum on the accumulate port).  Engines drain their
                #      queues in program order, so item 1's sums must be
                #      emitted here -- after item 0's SE/ubt work -- not up
                #      front, or they'd stall item 0's whole chain behind
                #      item 1's loads.  GpSimd/walrus support neither a
                #      free-dim reduce nor the accum rider. ----
                xp = spool.tile([P, KC], f32, tag="xp")
                for k in range(KC):
                    if b == 0:
                        nc.vector.reduce_sum(out=xp[:, k:k + 1],
                                             in_=xbs[b][k][:], axis=AX.X)
                    else:
                        nc.scalar.activation(out=xbs[b][k][:],
                                             in_=xbs[b][k][:],
                                             func=AF.Identity,
                                             accum_out=xp[:, k:k + 1])

                # ---- SE gate ----
                php = ps_se.tile([SE_C, 1], f32, tag="ps_se")
                for k in range(KC):
                    nc.tensor.matmul(php[:], w1t[:, k, :], xp[:, k:k + 1],
                                     start=(k == 0), stop=(k == KC - 1))
                hid = spool.tile([SE_C, 1], f32, tag="hid")
                nc.scalar.activation(out=hid[:], in_=php[:], func=AF.Relu,
                                     bias=b1t[:], scale=1.0)

                gate = spool.tile([P, KC], f32, tag="gate")
                for k in range(KC):
                    gp = ps_se.tile([P, 1], f32, tag="ps_se")
                    nc.tensor.matmul(gp[:], w2t[:, k * P:(k + 1) * P], hid[:],
                                     start=True, stop=True)
                    nc.scalar.activation(out=gate[:, k:k + 1], in_=gp[:],
                                         func=AF.Sigmoid, bias=b2t[:, k:k + 1],
                                         scale=1.0)

                # ---- gated projections: proj = w3e.T @ x (bf16 PE) ----
                w3e = spool.tile([P, KC, 3], bf16, tag="w3e")
                for k in range(KC):
                    nc.vector.tensor_scalar_mul(out=w3e[:, k, :],
                                                in0=w3t[:, k, :],
                                                scalar1=gate[:, k:k + 1])

                proj = ppool.tile([3, N], bf16, tag="proj")
                ubt = tpool.tile([P, N], bf16, tag="ubt")
                for j in range(NJ):
                    sl = slice(j * NB, (j + 1) * NB)
                    pp = ps_pp.tile([3, NB], f32, tag="pp")
                    for k in range(KC):
                        nc.tensor.matmul(pp[:], w3e[:, k, :], xbs[b][k][:, sl],
                                         start=(k == 0), stop=(k == KC - 1))
                    nc.vector.tensor_copy(out=proj[:, sl], in_=pp[:])
                    # theta -> all 128 partitions, on-chip: selector matmul
                    ub_ps = ps_ub.tile([P, NB], f32, tag="ub_ps")
                    nc.tensor.matmul(ub_ps[:], selt[:], proj[:, sl],
                                     start=True, stop=True)
                    nc.scalar.activation(out=ubt[:, sl], in_=ub_ps[:],
                                         func=AF.Identity, scale=1.0)

                # ---- g,phi -> [128, 36] (DRAM bounce) so the dot product
                #      uses every DVE lane ----
                g_rs = spool.tile([P, MR], bf16, tag="g_rs")
                p_rs = spool.tile([P, MR], bf16, tag="p_rs")
                nc.sync.dma_start(out=tp_scr[b], in_=proj[:])
                nc.sync.dma_start(
                    out=g_rs[:],
                    in_=tp_scr[b, 0, :].rearrange("(p m) -> p m", p=P))
                nc.sync.dma_start(
                    out=p_rs[:],
                    in_=tp_scr[b, 2, :].rearrange("(p m) -> p m", p=P))

                # ---- s_raw = <g, phi>; ast = A * s_raw ----
                prod = spool.tile([P, MR], f32, tag="prod")
                nc.vector.tensor_mul(out=prod[:], in0=g_rs[:], in1=p_rs[:])
                r1 = spool.tile([P, 1], f32, tag="r1")
                nc.vector.reduce_sum(out=r1[:], in_=prod[:], axis=AX.X)
                sb = ps_se.tile([P, 1], f32, tag="ps_se")
                nc.tensor.matmul(sb[:], on128[:], r1[:], start=True, stop=True)
                ast = spool.tile([P, KC], f32, tag="ast")
                nc.vector.tensor_scalar_mul(out=ast[:], in0=att[:],
                                            scalar1=sb[:])

                # ---- out = bf16(x) + (A*s)*theta + Bc, in place; store with
                #      bf16->f32 cast in the DMA.  Affine at 4x + add at 2x,
                #      both DVE (ACT is busy with item 1's sums here) ----
                for k in range(KC):
                    t1 = tpool.tile([P, N], bf16, tag="t1")
                    nc.vector.tensor_scalar(out=t1[:], in0=ubt[:],
                                            scalar1=ast[:, k:k + 1],
                                            scalar2=bct[:, k:k + 1],
                                            op0=ALU.mult, op1=ALU.add)
                    nc.vector.tensor_add(out=xbs[b][k][:],
                                         in0=xbs[b][k][:], in1=t1[:])
                    nc.gpsimd.dma_start(out=out_d[b, k * P:(k + 1) * P, :],
                                        in_=xbs[b][k][:])

    nc.finalize()  # runs Bacc compile passes (wait splitting, reg alloc, ...)
    return nc


def kernel(**inputs):
    global LAST_RESULTS
    from concourse.bass_utils import run_bass_kernel_spmd

    a = {k: np.asarray(v, dtype=np.float32) for k, v in inputs.items()}
    x = np.ascontiguousarray(a["x"]).reshape(B, C, N)

    inv = a["bn_gamma"] / np.sqrt(a["bn_var"] + BN_EPS)
    A = (a["W_w"] * inv / N).astype(np.float32)
    Bc = ((a["W_b"] - a["bn_mean"]) * inv + a["bn_beta"]).astype(np.float32)

    # the kernel folds the (always-zero) g/theta/phi conv biases away
    assert abs(float(a["g_b"])) < 1e-30 and abs(float(a["theta_b"])) < 1e-30 \
        and abs(float(a["phi_b"])) < 1e-30, "nonzero projection bias"

    w1h = np.ascontiguousarray(
        (a["se_w1"] / N).T.reshape(KC, P, SE_C).transpose(1, 0, 2)).astype(np.float32)
    w2h = np.ascontiguousarray(a["se_w2"].T).astype(np.float32)
    b1h = np.ascontiguousarray(a["se_b1"].reshape(SE_C, 1))
    b2h = np.ascontiguousarray(a["se_b2"].reshape(KC, P).T)
    import ml_dtypes
    w3h = np.ascontiguousarray(
        np.stack([a["g_w"], a["theta_w"], a["phi_w"]], axis=1)
        .reshape(KC, P, 3).transpose(1, 0, 2)).astype(ml_dtypes.bfloat16)
    ath = np.ascontiguousarray(A.reshape(KC, P).T)
    bch = np.ascontiguousarray(Bc.reshape(KC, P).T)
    selh = np.zeros((3, P), dtype=ml_dtypes.bfloat16)
    selh[1, :] = 1.0

    if "nc" not in _CACHE:
        _CACHE["nc"] = _build_bass()
    nc = _CACHE["nc"]

    in_maps = []
    for c in range(NCORES):
        in_maps.append({
            "xs": np.ascontiguousarray(x[c * BPC:(c + 1) * BPC]),
            "w1": w1h, "w2": w2h, "b1": b1h, "b2": b2h,
            "w3": w3h, "at": ath, "bc": bch, "sel": selh,
        })

    res = run_bass_kernel_spmd(nc, in_maps, core_ids=list(range(NCORES)))
    LAST_RESULTS = res

    out = np.concatenate([res.results[c]["out"] for c in range(NCORES)], axis=0)
    return np.ascontiguousarray(out.reshape(B, C, H, W))


# revision 25
# speedup vs baseline: 1.1216x; 1.0903x over previous
"""Trainium2 Bass kernel for the SE-gated Non-local block (rank-1 attention).

Math (per batch item b, x viewed as [C, N] with N = H*W):
    S[c]    = sum_n x[c, n]                      (spatial sum)
    hid     = relu((se_w1 / N) @ S + se_b1)      (SE bottleneck; 1/N folds the mean)
    gate    = sigmoid(se_w2 @ hid + se_b2)       [C]
    w3e     = gate * [g_w | theta_w | phi_w]     [C, 3]   (gate folded into projections)
    proj    = w3e.T @ x                          [3, N]   (rows: g, theta, phi;
                                                 the 1x1-conv biases are zero)
    s_raw   = sum_n proj[0] * proj[2]
    out     = x + (A * s_raw) (outer) theta + Bc (outer) ones   where
              inv = bn_gamma / sqrt(bn_var + eps)
              A   = W_w * inv / N                (1/N folds the f/N normalizer)
              Bc  = (W_b - bn_mean) * inv + bn_beta

Memory-roofline design: the only mandatory HBM traffic is read-x + write-out
(37.7 MB/core at 358 GB/s ~ 105 us).  To keep the DMA rings saturated the
whole time, x is held in SBUF as *bf16* (4.7 MB/item instead of 9.4), so both
batch items of a core fit on-chip and all 8 chunk loads stream back-to-back
while item 0's gate/projection chain overlaps item 1's loads, and item 0's
stores overlap item 1's chain.  The f32->bf16 cast happens inside the load
DMA (SWDGE) and the bf16->f32 cast inside the store DMA, so no engine pass
touches the bulk data except the single in-place correction add.

Precision: out = bf16(x) + correction, quantized to bf16 before the store
cast.  That costs ~2e-3 output rel err (bf16 mantissa), well inside the
2e-2 gate; the correction term itself (rms ~5e-6 vs |x| ~ 1) runs in bf16
end-to-end.

Engine budget (per item, vs the ~26 us load/store window it must hide in):
  DVE    ~18 us: item-0 spatial sums (1x reduce), proj psum combines,
                 w3e, dot, 2 affines (4x), 4 in-place adds (2x)
  ACT    ~12 us: SE relu/sigmoid, 9 ubt copies, 2 affines
  GpSimd       : item-1 spatial sums (streams behind the loads) + SWDGE
                 descriptor generation for the cast loads/stores
  PE     ~10 us: SE matmuls, proj (2 psum banks in parallel), theta
                 broadcast (selector matmul), cross-partition ones-matmul

Queue layout: x loads + out stores on the gpsimd SWDGE ring (they need the
dtype cast), weight loads + the g/phi row bounce on the sync HWDGE ring.
Sharding: pure data parallel, 2 of the 16 batch items per core, params
replicated, no collectives.
"""

import numpy as np

B, C, H, W = 16, 512, 96, 48
N = H * W            # 4608
P = 128
KC = C // P          # 4 channel chunks
NB = 512             # free-dim block = one fp32 PSUM bank
NJ = N // NB         # 9
NCORES = 8
BPC = B // NCORES    # 2 batch items per core
SE_C = C // 16       # 32
BN_EPS = 1e-5

_CACHE = {}
LAST_RESULTS = None


def _build_bass():
    import concourse.mybir as mybir
    from concourse.bacc import Bacc
    from concourse.tile import TileContext

    f32 = mybir.dt.float32
    bf16 = mybir.dt.bfloat16
    AF = mybir.ActivationFunctionType
    AX = mybir.AxisListType
    ALU = mybir.AluOpType

    nc = Bacc()
    xs = nc.dram_tensor("xs", [BPC, C, N], f32, kind="ExternalInput")
    w1 = nc.dram_tensor("w1", [P, KC, SE_C], f32, kind="ExternalInput")
    w2 = nc.dram_tensor("w2", [SE_C, C], f32, kind="ExternalInput")
    b1 = nc.dram_tensor("b1", [SE_C, 1], f32, kind="ExternalInput")
    b2 = nc.dram_tensor("b2", [P, KC], f32, kind="ExternalInput")
    w3 = nc.dram_tensor("w3", [P, KC, 3], bf16, kind="ExternalInput")
    at = nc.dram_tensor("at", [P, KC], f32, kind="ExternalInput")   # A chunks
    bc = nc.dram_tensor("bc", [P, KC], f32, kind="ExternalInput")   # Bc chunks
    sel = nc.dram_tensor("sel", [3, P], bf16, kind="ExternalInput")  # theta row
    out_d = nc.dram_tensor("out", [BPC, C, N], f32, kind="ExternalOutput")
    tp_scr = nc.dram_tensor("tp_scr", [BPC, 3, N], bf16)

    MR = N // P  # 36: elements per partition in the reshaped g/phi rows

    with TileContext(nc) as tc:
        with (
            tc.tile_pool(name="wpool", bufs=1) as wpool,
            tc.tile_pool(name="xpool", bufs=2 * KC) as xpool,
            tc.tile_pool(name="ppool", bufs=2) as ppool,
            tc.tile_pool(name="spool", bufs=2) as spool,
            tc.tile_pool(name="tpool", bufs=2) as tpool,
            tc.tile_pool(name="ps_se", bufs=2, space="PSUM") as ps_se,
            tc.tile_pool(name="ps_pp", bufs=3, space="PSUM") as ps_pp,
            tc.tile_pool(name="ps_ub", bufs=2, space="PSUM") as ps_ub,
        ):
            w1t = wpool.tile([P, KC, SE_C], f32, tag="w1t")
            w2t = wpool.tile([SE_C, C], f32, tag="w2t")
            b1t = wpool.tile([SE_C, 1], f32, tag="b1t")
            b2t = wpool.tile([P, KC], f32, tag="b2t")
            w3t = wpool.tile([P, KC, 3], bf16, tag="w3t")
            att = wpool.tile([P, KC], f32, tag="att")
            bct = wpool.tile([P, KC], f32, tag="bct")
            selt = wpool.tile([3, P], bf16, tag="selt")
            on128 = wpool.tile([P, P], f32, tag="on128")  # all-ones (part. sum)
            warm = wpool.tile([P, 1], f32, tag="warm")

            # weights on the sync ring so the gpsimd ring is free for x loads
            nc.vector.memset(on128[:], 1.0)
            # pull the sigmoid ACT table load out of the critical chain
            nc.scalar.activation(out=warm[:], in_=on128[:, 0:1],
                                 func=AF.Sigmoid)
            for t, d in ((w1t, w1), (w2t, w2), (b1t, b1), (b2t, b2),
                         (w3t, w3), (att, at), (bct, bc), (selt, sel)):
                nc.sync.dma_start(out=t[:], in_=d[:])

            # ---- all 8 x-chunk loads enqueued upfront (SWDGE f32->bf16) ----
            xbs = [[None] * KC for _ in range(BPC)]
            for b in range(BPC):
                for k in range(KC):
                    xb = xpool.tile([P, N], bf16, tag="xb")
                    nc.gpsimd.dma_start(out=xb[:], in_=xs[b, k * P:(k + 1) * P, :])
                    xbs[b][k] = xb

            # Per-engine queues drain in (scheduler-estimated) order, so the
            # emission sequence below hand-schedules the two items around
            # each other: item 1's sums live on ACT (emitted between item
            # 0's SE and item 1's SE), item 0's psum copies on DVE vs item
            # 1's on ACT (each lands on the engine that is idle at that
            # moment), and item 1's tiny w3e ops are emitted inside item
            # 0's output loop right where their gate becomes ready.

            def sums(b, xp):
                # spatial sums, streaming behind the loads
                for k in range(KC):
                    if b == 0:
                        nc.vector.reduce_sum(out=xp[:, k:k + 1],
                                             in_=xbs[b][k][:], axis=AX.X)
                    else:
                        # ACT: in-place identity, sum on the accumulate
                        # port (GpSimd/walrus support neither a free-dim
                        # reduce nor the accum rider; DVE is busy)
                        nc.scalar.activation(out=xbs[b][k][:],
                                             in_=xbs[b][k][:],
                                             func=AF.Identity,
                                             accum_out=xp[:, k:k + 1])

            def se_gate(b, xp):
                php = ps_se.tile([SE_C, 1], f32, tag="ps_se")
                for k in range(KC):
                    nc.tensor.matmul(php[:], w1t[:, k, :], xp[:, k:k + 1],
                                     start=(k == 0), stop=(k == KC - 1))
                hid = spool.tile([SE_C, 1], f32, tag="hid")
                nc.scalar.activation(out=hid[:], in_=php[:], func=AF.Relu,
                                     bias=b1t[:], scale=1.0)
                gate = spool.tile([P, KC], f32, tag="gate")
                for k in range(KC):
                    gp = ps_se.tile([P, 1], f32, tag="ps_se")
                    nc.tensor.matmul(gp[:], w2t[:, k * P:(k + 1) * P], hid[:],
                                     start=True, stop=True)
                    nc.scalar.activation(out=gate[:, k:k + 1], in_=gp[:],
                                         func=AF.Sigmoid, bias=b2t[:, k:k + 1],
                                         scale=1.0)
                return gate

            def w3e_of(gate):
                w3e = spool.tile([P, KC, 3], bf16, tag="w3e")
                for k in range(KC):
                    nc.vector.tensor_scalar_mul(out=w3e[:, k, :],
                                                in0=w3t[:, k, :],
                                                scalar1=gate[:, k:k + 1])
                return w3e

            def proj_phase(b, w3e):
                # gated projections proj = w3e.T @ x (bf16 PE) + theta
                # broadcast; psum->SBUF copies on DVE for item 0, ACT for
                # item 1 (the other engine is the busy one each time)
                proj = ppool.tile([3, N], bf16, tag="proj")
                ubt = tpool.tile([P, N], bf16, tag="ubt")
                for j in range(NJ):
                    sl = slice(j * NB, (j + 1) * NB)
                    pp = ps_pp.tile([3, NB], f32, tag="pp")
                    for k in range(KC):
                        nc.tensor.matmul(pp[:], w3e[:, k, :], xbs[b][k][:, sl],
                                         start=(k == 0), stop=(k == KC - 1))
                    ub_ps = ps_ub.tile([P, NB], f32, tag="ub_ps")
                    if b == 0:
                        nc.vector.tensor_copy(out=proj[:, sl], in_=pp[:])
                        nc.tensor.matmul(ub_ps[:], selt[:], proj[:, sl],
                                         start=True, stop=True)
                        nc.vector.tensor_copy(out=ubt[:, sl], in_=ub_ps[:])
                    else:
                        nc.scalar.activation(out=proj[:, sl], in_=pp[:],
                                             func=AF.Identity, scale=1.0)
                        nc.tensor.matmul(ub_ps[:], selt[:], proj[:, sl],
                                         start=True, stop=True)
                        nc.scalar.activation(out=ubt[:, sl], in_=ub_ps[:],
                                             func=AF.Identity, scale=1.0)
                return proj, ubt

            def dot_phase(b, proj):
                # g,phi -> [128, 36] via a DRAM bounce so the dot product
                # uses every DVE lane; cross-partition sum via ones-matmul
                g_rs = spool.tile([P, MR], bf16, tag="g_rs")
                p_rs = spool.tile([P, MR], bf16, tag="p_rs")
                nc.sync.dma_start(out=tp_scr[b], in_=proj[:])
                nc.sync.dma_start(
                    out=g_rs[:],
                    in_=tp_scr[b, 0, :].rearrange("(p m) -> p m", p=P))
                nc.sync.dma_start(
                    out=p_rs[:],
                    in_=tp_scr[b, 2, :].rearrange("(p m) -> p m", p=P))
                prod = spool.tile([P, MR], f32, tag="prod")
                nc.vector.tensor_mul(out=prod[:], in0=g_rs[:], in1=p_rs[:])
                r1 = spool.tile([P, 1], f32, tag="r1")
                nc.vector.reduce_sum(out=r1[:], in_=prod[:], axis=AX.X)
                sb = ps_se.tile([P, 1], f32, tag="ps_se")
                nc.tensor.matmul(sb[:], on128[:], r1[:], start=True, stop=True)
                ast = spool.tile([P, KC], f32, tag="ast")
                nc.vector.tensor_scalar_mul(out=ast[:], in0=att[:],
                                            scalar1=sb[:])
                return ast

            def out_chunk(b, k, ubt, ast):
                # out = bf16(x) + (A*s)*theta + Bc in place; store casts
                # bf16->f32 in the DMA.  Affine 4x + add 2x, both DVE.
                t1 = tpool.tile([P, N], bf16, tag="t1")
                nc.vector.tensor_scalar(out=t1[:], in0=ubt[:],
                                        scalar1=ast[:, k:k + 1],
                                        scalar2=bct[:, k:k + 1],
                                        op0=ALU.mult, op1=ALU.add)
                nc.vector.tensor_add(out=xbs[b][k][:],
                                     in0=xbs[b][k][:], in1=t1[:])
                nc.gpsimd.dma_start(out=out_d[b, k * P:(k + 1) * P, :],
                                    in_=xbs[b][k][:])

            xp0 = spool.tile([P, KC], f32, tag="xp")
            xp1 = spool.tile([P, KC], f32, tag="xp")

            sums(0, xp0)
            gate0 = se_gate(0, xp0)
            w3e0 = w3e_of(gate0)
            proj0, ubt0 = proj_phase(0, w3e0)
            ast0 = dot_phase(0, proj0)
            sums(1, xp1)              # ACT, load-gated; fills the gaps
            gate1 = se_gate(1, xp1)   # PE stalls on xp1 -- it's idle anyway
            out_chunk(0, 0, ubt0, ast0)
            out_chunk(0, 1, ubt0, ast0)
            w3e1 = w3e_of(gate1)      # mid-loop: right when gate1 is ready
            out_chunk(0, 2, ubt0, ast0)
            out_chunk(0, 3, ubt0, ast0)
            proj1, ubt1 = proj_phase(1, w3e1)
            ast1 = dot_phase(1, proj1)
            for k in range(KC):
                out_chunk(1, k, ubt1, ast1)

    nc.finalize()  # runs Bacc compile passes (wait splitting, reg alloc, ...)
    return nc


def kernel(**inputs):
    global LAST_RESULTS
    from concourse.bass_utils import run_bass_kernel_spmd

    a = {k: np.asarray(v, dtype=np.float32) for k, v in inputs.items()}
    x = np.ascontiguousarray(a["x"]).reshape(B, C, N)

    inv = a["bn_gamma"] / np.sqrt(a["bn_var"] + BN_EPS)
    A = (a["W_w"] * inv / N).astype(np.float32)
    Bc = ((a["W_b"] - a["bn_mean"]) * inv + a["bn_beta"]).astype(np.float32)

    # the kernel folds the (always-zero) g/theta/phi conv biases away
    assert abs(float(a["g_b"])) < 1e-30 and abs(float(a["theta_b"])) < 1e-30 \
        and abs(float(a["phi_b"])) < 1e-30, "nonzero projection bias"

    w1h = np.ascontiguousarray(
        (a["se_w1"] / N).T.reshape(KC, P, SE_C).transpose(1, 0, 2)).astype(np.float32)
    w2h = np.ascontiguousarray(a["se_w2"].T).astype(np.float32)
    b1h = np.ascontiguousarray(a["se_b1"].reshape(SE_C, 1))
    b2h = np.ascontiguousarray(a["se_b2"].reshape(KC, P).T)
    import ml_dtypes
    w3h = np.ascontiguousarray(
        np.stack([a["g_w"], a["theta_w"], a["phi_w"]], axis=1)
        .reshape(KC, P, 3).transpose(1, 0, 2)).astype(ml_dtypes.bfloat16)
    ath = np.ascontiguousarray(A.reshape(KC, P).T)
    bch = np.ascontiguousarray(Bc.reshape(KC, P).T)
    selh = np.zeros((3, P), dtype=ml_dtypes.bfloat16)
    selh[1, :] = 1.0

    if "nc" not in _CACHE:
        _CACHE["nc"] = _build_bass()
    nc = _CACHE["nc"]

    in_maps = []
    for c in range(NCORES):
        in_maps.append({
            "xs": np.ascontiguousarray(x[c * BPC:(c + 1) * BPC]),
            "w1": w1h, "w2": w2h, "b1": b1h, "b2": b2h,
            "w3": w3h, "at": ath, "bc": bch, "sel": selh,
        })

    res = run_bass_kernel_spmd(nc, in_maps, core_ids=list(range(NCORES)))
    LAST_RESULTS = res

    out = np.concatenate([res.results[c]["out"] for c in range(NCORES)], axis=0)
    return np.ascontiguousarray(out.reshape(B, C, H, W))


# revision 26
# speedup vs baseline: 1.1693x; 1.0425x over previous
"""Trainium2 Bass kernel for the SE-gated Non-local block (rank-1 attention).

Math (per batch item b, x viewed as [C, N] with N = H*W):
    S[c]    = sum_n x[c, n]                      (spatial sum)
    hid     = relu((se_w1 / N) @ S + se_b1)      (SE bottleneck; 1/N folds the mean)
    gate    = sigmoid(se_w2 @ hid + se_b2)       [C]
    w3e     = gate * [g_w | theta_w | phi_w]     [C, 3]   (gate folded into projections)
    proj    = w3e.T @ x                          [3, N]   (rows: g, theta, phi;
                                                 the 1x1-conv biases are zero)
    s_raw   = sum_n proj[0] * proj[2]
    out     = x + (A * s_raw) (outer) theta + Bc (outer) ones   where
              inv = bn_gamma / sqrt(bn_var + eps)
              A   = W_w * inv / N                (1/N folds the f/N normalizer)
              Bc  = (W_b - bn_mean) * inv + bn_beta

Memory-roofline design: the only mandatory HBM traffic is read-x + write-out
(37.7 MB/core at 358 GB/s ~ 105 us).  To keep the DMA rings saturated the
whole time, x is held in SBUF as *bf16* (4.7 MB/item instead of 9.4), so both
batch items of a core fit on-chip and all 8 chunk loads stream back-to-back
while item 0's gate/projection chain overlaps item 1's loads, and item 0's
stores overlap item 1's chain.  The f32->bf16 cast happens inside the load
DMA (SWDGE) and the bf16->f32 cast inside the store DMA, so no engine pass
touches the bulk data except the single in-place correction add.

Precision: out = bf16(x) + correction, quantized to bf16 before the store
cast.  That costs ~2e-3 output rel err (bf16 mantissa), well inside the
2e-2 gate; the correction term itself (rms ~5e-6 vs |x| ~ 1) runs in bf16
end-to-end.

Engine budget (per item, vs the ~26 us load/store window it must hide in):
  DVE    ~18 us: item-0 spatial sums (1x reduce), proj psum combines,
                 w3e, dot, 2 affines (4x), 4 in-place adds (2x)
  ACT    ~12 us: SE relu/sigmoid, 9 ubt copies, 2 affines
  GpSimd       : item-1 spatial sums (streams behind the loads) + SWDGE
                 descriptor generation for the cast loads/stores
  PE     ~10 us: SE matmuls, proj (2 psum banks in parallel), theta
                 broadcast (selector matmul), cross-partition ones-matmul

Queue layout: x loads + out stores on the gpsimd SWDGE ring (they need the
dtype cast), weight loads + the g/phi row bounce on the sync HWDGE ring.
Sharding: pure data parallel, 2 of the 16 batch items per core, params
replicated, no collectives.
"""

import numpy as np

B, C, H, W = 16, 512, 96, 48
N = H * W            # 4608
P = 128
KC = C // P          # 4 channel chunks
NB = 512             # free-dim block = one fp32 PSUM bank
NJ = N // NB         # 9
NCORES = 8
BPC = B // NCORES    # 2 batch items per core
SE_C = C // 16       # 32
BN_EPS = 1e-5

_CACHE = {}
LAST_RESULTS = None


def _build_bass():
    import concourse.mybir as mybir
    from concourse.bacc import Bacc
    from concourse.tile import TileContext

    f32 = mybir.dt.float32
    bf16 = mybir.dt.bfloat16
    AF = mybir.ActivationFunctionType
    AX = mybir.AxisListType
    ALU = mybir.AluOpType

    nc = Bacc()
    xs = nc.dram_tensor("xs", [BPC, C, N], f32, kind="ExternalInput")
    w1 = nc.dram_tensor("w1", [P, KC, SE_C], f32, kind="ExternalInput")
    w2 = nc.dram_tensor("w2", [SE_C, C], f32, kind="ExternalInput")
    b1 = nc.dram_tensor("b1", [SE_C, 1], f32, kind="ExternalInput")
    b2 = nc.dram_tensor("b2", [P, KC], f32, kind="ExternalInput")
    w3 = nc.dram_tensor("w3", [P, KC, 3], bf16, kind="ExternalInput")
    at = nc.dram_tensor("at", [P, KC], f32, kind="ExternalInput")   # A chunks
    bc = nc.dram_tensor("bc", [P, KC], f32, kind="ExternalInput")   # Bc chunks
    sel = nc.dram_tensor("sel", [3, P], bf16, kind="ExternalInput")  # theta row
    out_d = nc.dram_tensor("out", [BPC, C, N], f32, kind="ExternalOutput")
    tp_scr = nc.dram_tensor("tp_scr", [BPC, 3, N], bf16)

    MR = N // P  # 36: elements per partition in the reshaped g/phi rows

    with TileContext(nc) as tc:
        with (
            tc.tile_pool(name="wpool", bufs=1) as wpool,
            tc.tile_pool(name="xpool", bufs=2 * KC) as xpool,
            tc.tile_pool(name="ppool", bufs=2) as ppool,
            tc.tile_pool(name="spool", bufs=2) as spool,
            tc.tile_pool(name="tpool", bufs=2) as tpool,
            tc.tile_pool(name="ps_se", bufs=2, space="PSUM") as ps_se,
            tc.tile_pool(name="ps_pp", bufs=3, space="PSUM") as ps_pp,
            tc.tile_pool(name="ps_ub", bufs=2, space="PSUM") as ps_ub,
        ):
            w1t = wpool.tile([P, KC, SE_C], f32, tag="w1t")
            w2t = wpool.tile([SE_C, C], f32, tag="w2t")
            b1t = wpool.tile([SE_C, 1], f32, tag="b1t")
            b2t = wpool.tile([P, KC], f32, tag="b2t")
            w3t = wpool.tile([P, KC, 3], bf16, tag="w3t")
            att = wpool.tile([P, KC], f32, tag="att")
            bct = wpool.tile([P, KC], f32, tag="bct")
            selt = wpool.tile([3, P], bf16, tag="selt")
            on128 = wpool.tile([P, P], f32, tag="on128")  # all-ones (part. sum)
            warm = wpool.tile([P, 1], f32, tag="warm")

            # weights on the sync ring so the gpsimd ring is free for x loads
            nc.vector.memset(on128[:], 1.0)
            # pull the sigmoid ACT table load out of the critical chain
            nc.scalar.activation(out=warm[:], in_=on128[:, 0:1],
                                 func=AF.Sigmoid)
            for t, d in ((w1t, w1), (w2t, w2), (b1t, b1), (b2t, b2),
                         (w3t, w3), (att, at), (bct, bc), (selt, sel)):
                nc.sync.dma_start(out=t[:], in_=d[:])

            # ---- all 8 x-chunk loads enqueued upfront (SWDGE f32->bf16) ----
            xbs = [[None] * KC for _ in range(BPC)]
            for b in range(BPC):
                for k in range(KC):
                    xb = xpool.tile([P, N], bf16, tag="xb")
                    nc.gpsimd.dma_start(out=xb[:], in_=xs[b, k * P:(k + 1) * P, :])
                    xbs[b][k] = xb

            # Per-engine queues drain in (scheduler-estimated) order, so the
            # emission sequence below hand-schedules the two items around
            # each other: item 1's sums live on ACT (emitted between item
            # 0's SE and item 1's SE), item 0's psum copies on DVE vs item
            # 1's on ACT (each lands on the engine that is idle at that
            # moment), and item 1's tiny w3e ops are emitted inside item
            # 0's output loop right where their gate becomes ready.

            def sums(b, xp):
                # spatial sums, streaming behind the loads
                for k in range(KC):
                    if b == 0:
                        nc.vector.reduce_sum(out=xp[:, k:k + 1],
                                             in_=xbs[b][k][:], axis=AX.X)
                    else:
                        # ACT: in-place identity, sum on the accumulate
                        # port (GpSimd/walrus support neither a free-dim
                        # reduce nor the accum rider; DVE is busy)
                        nc.scalar.activation(out=xbs[b][k][:],
                                             in_=xbs[b][k][:],
                                             func=AF.Identity,
                                             accum_out=xp[:, k:k + 1])

            def se_gate(b, xp):
                php = ps_se.tile([SE_C, 1], f32, tag="ps_se")
                for k in range(KC):
                    nc.tensor.matmul(php[:], w1t[:, k, :], xp[:, k:k + 1],
                                     start=(k == 0), stop=(k == KC - 1))
                hid = spool.tile([SE_C, 1], f32, tag="hid")
                nc.scalar.activation(out=hid[:], in_=php[:], func=AF.Relu,
                                     bias=b1t[:], scale=1.0)
                gate = spool.tile([P, KC], f32, tag="gate")
                for k in range(KC):
                    gp = ps_se.tile([P, 1], f32, tag="ps_se")
                    nc.tensor.matmul(gp[:], w2t[:, k * P:(k + 1) * P], hid[:],
                                     start=True, stop=True)
                    nc.scalar.activation(out=gate[:, k:k + 1], in_=gp[:],
                                         func=AF.Sigmoid, bias=b2t[:, k:k + 1],
                                         scale=1.0)
                return gate

            def w3e_of(gate):
                w3e = spool.tile([P, KC, 3], bf16, tag="w3e")
                for k in range(KC):
                    nc.vector.tensor_scalar_mul(out=w3e[:, k, :],
                                                in0=w3t[:, k, :],
                                                scalar1=gate[:, k:k + 1])
                return w3e

            def proj_phase(b, w3e):
                # gated projections proj = w3e.T @ x (bf16 PE) + theta
                # broadcast; psum->SBUF copies on DVE for item 0, ACT for
                # item 1 (the other engine is the busy one each time)
                proj = ppool.tile([3, N], bf16, tag="proj")
                ubt = tpool.tile([P, N], bf16, tag="ubt")
                for j in range(NJ):
                    sl = slice(j * NB, (j + 1) * NB)
                    pp = ps_pp.tile([3, NB], f32, tag="pp")
                    for k in range(KC):
                        nc.tensor.matmul(pp[:], w3e[:, k, :], xbs[b][k][:, sl],
                                         start=(k == 0), stop=(k == KC - 1))
                    ub_ps = ps_ub.tile([P, NB], f32, tag="ub_ps")
                    if b == 0:
                        nc.vector.tensor_copy(out=proj[:, sl], in_=pp[:])
                        nc.tensor.matmul(ub_ps[:], selt[:], proj[:, sl],
                                         start=True, stop=True)
                        nc.vector.tensor_copy(out=ubt[:, sl], in_=ub_ps[:])
                    else:
                        nc.scalar.activation(out=proj[:, sl], in_=pp[:],
                                             func=AF.Identity, scale=1.0)
                        nc.tensor.matmul(ub_ps[:], selt[:], proj[:, sl],
                                         start=True, stop=True)
                        nc.scalar.activation(out=ubt[:, sl], in_=ub_ps[:],
                                             func=AF.Identity, scale=1.0)
                return proj, ubt

            def dot_phase(b, proj):
                # g,phi -> [128, 36] via a DRAM bounce so the dot product
                # uses every DVE lane; cross-partition sum via ones-matmul
                g_rs = spool.tile([P, MR], bf16, tag="g_rs")
                p_rs = spool.tile([P, MR], bf16, tag="p_rs")
                nc.sync.dma_start(out=tp_scr[b], in_=proj[:])
                nc.sync.dma_start(
                    out=g_rs[:],
                    in_=tp_scr[b, 0, :].rearrange("(p m) -> p m", p=P))
                nc.sync.dma_start(
                    out=p_rs[:],
                    in_=tp_scr[b, 2, :].rearrange("(p m) -> p m", p=P))
                prod = spool.tile([P, MR], f32, tag="prod")
                nc.vector.tensor_mul(out=prod[:], in0=g_rs[:], in1=p_rs[:])
                r1 = spool.tile([P, 1], f32, tag="r1")
                nc.vector.reduce_sum(out=r1[:], in_=prod[:], axis=AX.X)
                sb = ps_se.tile([P, 1], f32, tag="ps_se")
                nc.tensor.matmul(sb[:], on128[:], r1[:], start=True, stop=True)
                ast = spool.tile([P, KC], f32, tag="ast")
                nc.vector.tensor_scalar_mul(out=ast[:], in0=att[:],
                                            scalar1=sb[:])
                return ast

            def out_chunk(b, k, ubt, ast):
                # out = bf16(x) + (A*s)*theta + Bc in place; store casts
                # bf16->f32 in the DMA.  Affine 4x + add 2x, both DVE.
                t1 = tpool.tile([P, N], bf16, tag="t1")
                nc.vector.tensor_scalar(out=t1[:], in0=ubt[:],
                                        scalar1=ast[:, k:k + 1],
                                        scalar2=bct[:, k:k + 1],
                                        op0=ALU.mult, op1=ALU.add)
                nc.vector.tensor_add(out=xbs[b][k][:],
                                     in0=xbs[b][k][:], in1=t1[:])
                nc.gpsimd.dma_start(out=out_d[b, k * P:(k + 1) * P, :],
                                    in_=xbs[b][k][:])

            xp0 = spool.tile([P, KC], f32, tag="xp")
            xp1 = spool.tile([P, KC], f32, tag="xp")

            sums(0, xp0)
            gate0 = se_gate(0, xp0)
            w3e0 = w3e_of(gate0)
            proj0, ubt0 = proj_phase(0, w3e0)
            ast0 = dot_phase(0, proj0)
            # The static scheduler orders each engine's queue by its own
            # (optimistic) ready-time estimates; without the logical-
            # timestamp floor it slots these 4.1us sums ahead of item 0's
            # relu/sigmoid and stalls item 0's whole chain ~10us.  50us is
            # past any estimate of item 0's SE but before item 1's last
            # load actually lands, so it cannot slow the real schedule.
            with tc.tile_wait_until(0.05):
                sums(1, xp1)          # ACT, load-gated; fills the gaps
            gate1 = se_gate(1, xp1)   # PE stalls on xp1 -- it's idle anyway
            out_chunk(0, 0, ubt0, ast0)
            out_chunk(0, 1, ubt0, ast0)
            w3e1 = w3e_of(gate1)      # mid-loop: right when gate1 is ready
            out_chunk(0, 2, ubt0, ast0)
            out_chunk(0, 3, ubt0, ast0)
            proj1, ubt1 = proj_phase(1, w3e1)
            ast1 = dot_phase(1, proj1)
            for k in range(KC):
                out_chunk(1, k, ubt1, ast1)

    nc.finalize()  # runs Bacc compile passes (wait splitting, reg alloc, ...)
    return nc


def kernel(**inputs):
    global LAST_RESULTS
    from concourse.bass_utils import run_bass_kernel_spmd

    a = {k: np.asarray(v, dtype=np.float32) for k, v in inputs.items()}
    x = np.ascontiguousarray(a["x"]).reshape(B, C, N)

    inv = a["bn_gamma"] / np.sqrt(a["bn_var"] + BN_EPS)
    A = (a["W_w"] * inv / N).astype(np.float32)
    Bc = ((a["W_b"] - a["bn_mean"]) * inv + a["bn_beta"]).astype(np.float32)

    # the kernel folds the (always-zero) g/theta/phi conv biases away
    assert abs(float(a["g_b"])) < 1e-30 and abs(float(a["theta_b"])) < 1e-30 \
        and abs(float(a["phi_b"])) < 1e-30, "nonzero projection bias"

    w1h = np.ascontiguousarray(
        (a["se_w1"] / N).T.reshape(KC, P, SE_C).transpose(1, 0, 2)).astype(np.float32)
    w2h = np.ascontiguousarray(a["se_w2"].T).astype(np.float32)
    b1h = np.ascontiguousarray(a["se_b1"].reshape(SE_C, 1))
    b2h = np.ascontiguousarray(a["se_b2"].reshape(KC, P).T)
    import ml_dtypes
    w3h = np.ascontiguousarray(
        np.stack([a["g_w"], a["theta_w"], a["phi_w"]], axis=1)
        .reshape(KC, P, 3).transpose(1, 0, 2)).astype(ml_dtypes.bfloat16)
    ath = np.ascontiguousarray(A.reshape(KC, P).T)
    bch = np.ascontiguousarray(Bc.reshape(KC, P).T)
    selh = np.zeros((3, P), dtype=ml_dtypes.bfloat16)
    selh[1, :] = 1.0

    if "nc" not in _CACHE:
        _CACHE["nc"] = _build_bass()
    nc = _CACHE["nc"]

    in_maps = []
    for c in range(NCORES):
        in_maps.append({
            "xs": np.ascontiguousarray(x[c * BPC:(c + 1) * BPC]),
            "w1": w1h, "w2": w2h, "b1": b1h, "b2": b2h,
            "w3": w3h, "at": ath, "bc": bch, "sel": selh,
        })

    res = run_bass_kernel_spmd(nc, in_maps, core_ids=list(range(NCORES)))
    LAST_RESULTS = res

    out = np.concatenate([res.results[c]["out"] for c in range(NCORES)], axis=0)
    return np.ascontiguousarray(out.reshape(B, C, H, W))


# revision 29
# speedup vs baseline: 1.1845x; 1.0130x over previous
"""Trainium2 Bass kernel for the SE-gated Non-local block (rank-1 attention).

Math (per batch item b, x viewed as [C, N] with N = H*W):
    S[c]    = sum_n x[c, n]                      (spatial sum)
    hid     = relu((se_w1 / N) @ S + se_b1)      (SE bottleneck; 1/N folds the mean)
    gate    = sigmoid(se_w2 @ hid + se_b2)       [C]
    w3e     = gate * [g_w | theta_w | phi_w]     [C, 3]   (gate folded into projections)
    proj    = w3e.T @ x                          [3, N]   (rows: g, theta, phi;
                                                 the 1x1-conv biases are zero)
    s_raw   = sum_n proj[0] * proj[2]
    out     = x + (A * s_raw) (outer) theta + Bc (outer) ones   where
              inv = bn_gamma / sqrt(bn_var + eps)
              A   = W_w * inv / N                (1/N folds the f/N normalizer)
              Bc  = (W_b - bn_mean) * inv + bn_beta

Memory-roofline design: the only mandatory HBM traffic is read-x + write-out
(37.7 MB/core at 358 GB/s ~ 105 us).  To keep the DMA rings saturated the
whole time, x is held in SBUF as *bf16* (4.7 MB/item instead of 9.4), so both
batch items of a core fit on-chip and all 8 chunk loads stream back-to-back
while item 0's gate/projection chain overlaps item 1's loads, and item 0's
stores overlap item 1's chain.  The f32->bf16 cast happens inside the load
DMA (SWDGE) and the bf16->f32 cast inside the store DMA, so no engine pass
touches the bulk data except the single in-place correction add.

Precision: out = bf16(x) + correction, quantized to bf16 before the store
cast.  That costs ~2e-3 output rel err (bf16 mantissa), well inside the
2e-2 gate; the correction term itself (rms ~5e-6 vs |x| ~ 1) runs in bf16
end-to-end.

Engine budget (per item, vs the ~26 us load/store window it must hide in):
  DVE    ~18 us: item-0 spatial sums (1x reduce), proj psum combines,
                 w3e, dot, 2 affines (4x), 4 in-place adds (2x)
  ACT    ~12 us: SE relu/sigmoid, 9 ubt copies, 2 affines
  GpSimd       : item-1 spatial sums (streams behind the loads) + SWDGE
                 descriptor generation for the cast loads/stores
  PE     ~10 us: SE matmuls, proj (2 psum banks in parallel), theta
                 broadcast (selector matmul), cross-partition ones-matmul

Queue layout: x loads + out stores on the gpsimd SWDGE ring (they need the
dtype cast), weight loads + the g/phi row bounce on the sync HWDGE ring.
Sharding: pure data parallel, 2 of the 16 batch items per core, params
replicated, no collectives.
"""

import numpy as np

B, C, H, W = 16, 512, 96, 48
N = H * W            # 4608
P = 128
KC = C // P          # 4 channel chunks
NB = 512             # free-dim block = one fp32 PSUM bank
NJ = N // NB         # 9
NCORES = 8
BPC = B // NCORES    # 2 batch items per core
SE_C = C // 16       # 32
BN_EPS = 1e-5

_CACHE = {}
LAST_RESULTS = None


def _build_bass():
    import concourse.mybir as mybir
    from concourse.bacc import Bacc
    from concourse.tile import TileContext

    f32 = mybir.dt.float32
    bf16 = mybir.dt.bfloat16
    AF = mybir.ActivationFunctionType
    AX = mybir.AxisListType
    ALU = mybir.AluOpType

    nc = Bacc()
    xs = nc.dram_tensor("xs", [BPC, C, N], f32, kind="ExternalInput")
    w1 = nc.dram_tensor("w1", [P, KC, SE_C], f32, kind="ExternalInput")
    w2 = nc.dram_tensor("w2", [SE_C, C], f32, kind="ExternalInput")
    b1 = nc.dram_tensor("b1", [SE_C, 1], f32, kind="ExternalInput")
    b2 = nc.dram_tensor("b2", [P, KC], f32, kind="ExternalInput")
    w3 = nc.dram_tensor("w3", [P, KC, 3], bf16, kind="ExternalInput")
    at = nc.dram_tensor("at", [P, KC], f32, kind="ExternalInput")   # A chunks
    bc = nc.dram_tensor("bc", [P, KC], f32, kind="ExternalInput")   # Bc chunks
    sel = nc.dram_tensor("sel", [3, P], bf16, kind="ExternalInput")  # theta row
    out_d = nc.dram_tensor("out", [BPC, C, N], f32, kind="ExternalOutput")
    tp_scr = nc.dram_tensor("tp_scr", [BPC, 3, N], bf16)

    MR = N // P  # 36: elements per partition in the reshaped g/phi rows

    with TileContext(nc) as tc:
        with (
            tc.tile_pool(name="wpool", bufs=1) as wpool,
            tc.tile_pool(name="xpool", bufs=2 * KC) as xpool,
            tc.tile_pool(name="ppool", bufs=2) as ppool,
            tc.tile_pool(name="spool", bufs=2) as spool,
            tc.tile_pool(name="tpool", bufs=2) as tpool,
            tc.tile_pool(name="ps_se", bufs=2, space="PSUM") as ps_se,
            tc.tile_pool(name="ps_pp", bufs=3, space="PSUM") as ps_pp,
            tc.tile_pool(name="ps_ub", bufs=2, space="PSUM") as ps_ub,
        ):
            w1t = wpool.tile([P, KC, SE_C], f32, tag="w1t")
            w2t = wpool.tile([SE_C, C], f32, tag="w2t")
            b1t = wpool.tile([SE_C, 1], f32, tag="b1t")
            b2t = wpool.tile([P, KC], f32, tag="b2t")
            w3t = wpool.tile([P, KC, 3], bf16, tag="w3t")
            att = wpool.tile([P, KC], f32, tag="att")
            bct = wpool.tile([P, KC], f32, tag="bct")
            selt = wpool.tile([3, P], bf16, tag="selt")
            on128 = wpool.tile([P, P], f32, tag="on128")  # all-ones (part. sum)
            warm = wpool.tile([P, 1], f32, tag="warm")

            # weights on the sync ring so the gpsimd ring is free for x loads
            nc.vector.memset(on128[:], 1.0)
            # pull the sigmoid ACT table load out of the critical chain
            nc.scalar.activation(out=warm[:], in_=on128[:, 0:1],
                                 func=AF.Sigmoid)
            for t, d in ((w1t, w1), (w2t, w2), (b1t, b1), (b2t, b2),
                         (w3t, w3), (att, at), (bct, bc), (selt, sel)):
                nc.sync.dma_start(out=t[:], in_=d[:])

            # ---- all 8 x-chunk loads enqueued upfront (SWDGE f32->bf16) ----
            xbs = [[None] * KC for _ in range(BPC)]
            for b in range(BPC):
                for k in range(KC):
                    xb = xpool.tile([P, N], bf16, tag="xb")
                    nc.gpsimd.dma_start(out=xb[:], in_=xs[b, k * P:(k + 1) * P, :])
                    xbs[b][k] = xb

            # Per-engine queues drain in (scheduler-estimated) order, so the
            # emission sequence below hand-schedules the two items around
            # each other: item 1's sums live on ACT (emitted between item
            # 0's SE and item 1's SE), item 0's psum copies on DVE vs item
            # 1's on ACT (each lands on the engine that is idle at that
            # moment), and item 1's tiny w3e ops are emitted inside item
            # 0's output loop right where their gate becomes ready.

            def sums(b, xp):
                # spatial sums, streaming behind the loads
                for k in range(KC):
                    if b == 0:
                        nc.vector.reduce_sum(out=xp[:, k:k + 1],
                                             in_=xbs[b][k][:], axis=AX.X)
                    else:
                        # ACT: in-place identity, sum on the accumulate
                        # port (GpSimd/walrus support neither a free-dim
                        # reduce nor the accum rider; DVE is busy)
                        nc.scalar.activation(out=xbs[b][k][:],
                                             in_=xbs[b][k][:],
                                             func=AF.Identity,
                                             accum_out=xp[:, k:k + 1])

            def se_gate(b, xp):
                php = ps_se.tile([SE_C, 1], f32, tag="ps_se")
                for k in range(KC):
                    nc.tensor.matmul(php[:], w1t[:, k, :], xp[:, k:k + 1],
                                     start=(k == 0), stop=(k == KC - 1))
                hid = spool.tile([SE_C, 1], f32, tag="hid")
                nc.scalar.activation(out=hid[:], in_=php[:], func=AF.Relu,
                                     bias=b1t[:], scale=1.0)
                gate = spool.tile([P, KC], f32, tag="gate")
                for k in range(KC):
                    gp = ps_se.tile([P, 1], f32, tag="ps_se")
                    nc.tensor.matmul(gp[:], w2t[:, k * P:(k + 1) * P], hid[:],
                                     start=True, stop=True)
                    nc.scalar.activation(out=gate[:, k:k + 1], in_=gp[:],
                                         func=AF.Sigmoid, bias=b2t[:, k:k + 1],
                                         scale=1.0)
                return gate

            def w3e_of(gate):
                # on ACT (per-partition scale port): DVE may be mid-way
                # through the other item's output adds when gate lands
                w3e = spool.tile([P, KC, 3], bf16, tag="w3e")
                for k in range(KC):
                    nc.scalar.activation(out=w3e[:, k, :], in_=w3t[:, k, :],
                                         func=AF.Identity,
                                         scale=gate[:, k:k + 1])
                return w3e

            def proj_phase(b, w3e):
                # gated projections proj = w3e.T @ x (bf16 PE) + theta
                # broadcast; psum->SBUF copies on DVE for item 0, ACT for
                # item 1 (the other engine is the busy one each time)
                proj = ppool.tile([3, N], bf16, tag="proj")
                ubt = tpool.tile([P, N], bf16, tag="ubt")
                for j in range(NJ):
                    sl = slice(j * NB, (j + 1) * NB)
                    pp = ps_pp.tile([3, NB], f32, tag="pp")
                    for k in range(KC):
                        nc.tensor.matmul(pp[:], w3e[:, k, :], xbs[b][k][:, sl],
                                         start=(k == 0), stop=(k == KC - 1))
                    ub_ps = ps_ub.tile([P, NB], f32, tag="ub_ps")
                    if b == 0:
                        # DVE proj copy + ACT ubt copy: the j-loop then
                        # paces on the PE, not on one copy engine
                        nc.vector.tensor_copy(out=proj[:, sl], in_=pp[:])
                    else:
                        nc.scalar.activation(out=proj[:, sl], in_=pp[:],
                                             func=AF.Identity, scale=1.0)
                    nc.tensor.matmul(ub_ps[:], selt[:], proj[:, sl],
                                     start=True, stop=True)
                    nc.scalar.activation(out=ubt[:, sl], in_=ub_ps[:],
                                         func=AF.Identity, scale=1.0)
                return proj, ubt

            def dot_phase(b, proj):
                # g,phi -> [128, 2, 36] via a DRAM bounce (one combined
                # readback) so the dot product uses every DVE lane;
                # cross-partition sum via ones-matmul
                gp_rs = spool.tile([P, 2, MR], bf16, tag="gp_rs")
                nc.sync.dma_start(out=tp_scr[b], in_=proj[:])
                nc.sync.dma_start(
                    out=gp_rs[:],
                    in_=tp_scr[b, 0:3:2, :].rearrange("r (p m) -> p r m",
                                                      p=P))
                prod = spool.tile([P, MR], f32, tag="prod")
                nc.vector.tensor_mul(out=prod[:], in0=gp_rs[:, 0, :],
                                     in1=gp_rs[:, 1, :])
                r1 = spool.tile([P, 1], f32, tag="r1")
                nc.vector.reduce_sum(out=r1[:], in_=prod[:], axis=AX.X)
                sb = ps_se.tile([P, 1], f32, tag="ps_se")
                nc.tensor.matmul(sb[:], on128[:], r1[:], start=True, stop=True)
                ast = spool.tile([P, KC], f32, tag="ast")
                nc.vector.tensor_scalar_mul(out=ast[:], in0=att[:],
                                            scalar1=sb[:])
                return ast

            def out_chunk(b, k, ubt, ast):
                # out = bf16(x) + (A*s)*theta + Bc in place; store casts
                # bf16->f32 in the DMA.  Affine 4x + add 2x, both DVE.
                t1 = tpool.tile([P, N], bf16, tag="t1")
                nc.vector.tensor_scalar(out=t1[:], in0=ubt[:],
                                        scalar1=ast[:, k:k + 1],
                                        scalar2=bct[:, k:k + 1],
                                        op0=ALU.mult, op1=ALU.add)
                nc.vector.tensor_add(out=xbs[b][k][:],
                                     in0=xbs[b][k][:], in1=t1[:])
                nc.gpsimd.dma_start(out=out_d[b, k * P:(k + 1) * P, :],
                                    in_=xbs[b][k][:])

            xp0 = spool.tile([P, KC], f32, tag="xp")
            xp1 = spool.tile([P, KC], f32, tag="xp")

            sums(0, xp0)
            gate0 = se_gate(0, xp0)
            w3e0 = w3e_of(gate0)
            proj0, ubt0 = proj_phase(0, w3e0)
            ast0 = dot_phase(0, proj0)
            # The static scheduler orders each engine's queue by its own
            # (optimistic) ready-time estimates; without the logical-
            # timestamp floor it slots these 4.1us sums ahead of item 0's
            # relu/sigmoid and stalls item 0's whole chain ~10us.  50us is
            # past any estimate of item 0's SE but before item 1's last
            # load actually lands, so it cannot slow the real schedule.
            with tc.tile_wait_until(0.05):
                sums(1, xp1)          # ACT, load-gated; fills the gaps
            gate1 = se_gate(1, xp1)   # PE stalls on xp1 -- it's idle anyway
            out_chunk(0, 0, ubt0, ast0)
            out_chunk(0, 1, ubt0, ast0)
            w3e1 = w3e_of(gate1)      # mid-loop: right when gate1 is ready
            out_chunk(0, 2, ubt0, ast0)
            out_chunk(0, 3, ubt0, ast0)
            proj1, ubt1 = proj_phase(1, w3e1)
            ast1 = dot_phase(1, proj1)
            for k in range(KC):
                out_chunk(1, k, ubt1, ast1)

    nc.finalize()  # runs Bacc compile passes (wait splitting, reg alloc, ...)
    return nc


def kernel(**inputs):
    global LAST_RESULTS
    from concourse.bass_utils import run_bass_kernel_spmd

    a = {k: np.asarray(v, dtype=np.float32) for k, v in inputs.items()}
    x = np.ascontiguousarray(a["x"]).reshape(B, C, N)

    inv = a["bn_gamma"] / np.sqrt(a["bn_var"] + BN_EPS)
    A = (a["W_w"] * inv / N).astype(np.float32)
    Bc = ((a["W_b"] - a["bn_mean"]) * inv + a["bn_beta"]).astype(np.float32)

    # the kernel folds the (always-zero) g/theta/phi conv biases away
    assert abs(float(a["g_b"])) < 1e-30 and abs(float(a["theta_b"])) < 1e-30 \
        and abs(float(a["phi_b"])) < 1e-30, "nonzero projection bias"

    w1h = np.ascontiguousarray(
        (a["se_w1"] / N).T.reshape(KC, P, SE_C).transpose(1, 0, 2)).astype(np.float32)
    w2h = np.ascontiguousarray(a["se_w2"].T).astype(np.float32)
    b1h = np.ascontiguousarray(a["se_b1"].reshape(SE_C, 1))
    b2h = np.ascontiguousarray(a["se_b2"].reshape(KC, P).T)
    import ml_dtypes
    w3h = np.ascontiguousarray(
        np.stack([a["g_w"], a["theta_w"], a["phi_w"]], axis=1)
        .reshape(KC, P, 3).transpose(1, 0, 2)).astype(ml_dtypes.bfloat16)
    ath = np.ascontiguousarray(A.reshape(KC, P).T)
    bch = np.ascontiguousarray(Bc.reshape(KC, P).T)
    selh = np.zeros((3, P), dtype=ml_dtypes.bfloat16)
    selh[1, :] = 1.0

    if "nc" not in _CACHE:
        _CACHE["nc"] = _build_bass()
    nc = _CACHE["nc"]

    in_maps = []
    for c in range(NCORES):
        in_maps.append({
            "xs": np.ascontiguousarray(x[c * BPC:(c + 1) * BPC]),
            "w1": w1h, "w2": w2h, "b1": b1h, "b2": b2h,
            "w3": w3h, "at": ath, "bc": bch, "sel": selh,
        })

    res = run_bass_kernel_spmd(nc, in_maps, core_ids=list(range(NCORES)))
    LAST_RESULTS = res

    out = np.concatenate([res.results[c]["out"] for c in range(NCORES)], axis=0)
    return np.ascontiguousarray(out.reshape(B, C, H, W))
